# revision 1
# baseline (speedup 1.0000x reference)
"""Sequence-parallel fused LayerNorm + QKV-projection + attention for TRN2.

x [8192,10] f32 -> LN -> h @ W.T -> q,k,v -> out = softmax(q k^T) v [8192,11].
The 8192 query rows are sharded across 8 NeuronCores (1024 each); every core
computes k/v for all keys itself (projection is tiny, no collectives).

Key design points (vs the 117.7us v1 baseline; this version ~87us):
- exp(sim) split across BOTH engines, one tile each per 2-tile batch:
  ACT uses the exp LUT; DVE uses a Schraudolph bit-trick (i16 =
  round(sim*1024*log2e + 15*1024 - C), bitcast to fp16; round-to-nearest
  int16 convert verified on HW; C=61 tuned offline; sawtooth error ~1.5%
  mostly cancels in the softmax ratio -> rel err ~9e-4).
- keys packed 4 rows/column-group at partition bases {0,32,64,96}; qk h0
  reads kvT on band b, qk h1 reads a 64-partition-rotated copy (kvT2) on
  band (b+2)%4, so all 4 qk matmuls of a batch run concurrently on
  distinct (row-band, PSUM-bank) pairs. NOTE: concurrent row-tiled MMs
  into the SAME PSUM bank fault the device.
- av halves spread over all 4 col groups via (t + 2h) % 4 (every group
  covers both column halves -- required, PSUM is stale where never
  written); merged in the epilogue after a transpose.
- v row-major obtained by tiled DMA xbar transposes ([32,1024] fp16 ->
  [128,8,32]) on the sync/scalar queues; explicit sync add_dep_helper
  edges on both sides (the xbar DMA's automatic dep tracking races).
- prologue pipelined in 16-row slices (stats -> Sqrt+reciprocal rsig ->
  normalize fp16 -> fp16 transposes -> fp16 projection), all four slices
  in the prologue (deferring them mid-loop stole sim-pool PSUM slots and
  cost ~2.5us); only the second v half is emitted mid-loop; ACT table
  sets (sqrt, exp) are each loaded exactly once via dummy warm-up ops.
"""

import ml_dtypes
import numpy as np

import concourse.bass as bass
import concourse.bacc as bacc
from concourse import mybir
from concourse.tile import TileContext
from concourse.tile_rust import add_dep_helper
from concourse.bass_utils import run_bass_kernel_spmd

F32 = mybir.dt.float32
FP16 = mybir.dt.float16
I16 = mybir.dt.int16

N = 8192
NCORES = 8
NQ = N // NCORES
P = 128
R = N // P
RQ = NQ // P
D = 10
DA = D + 1
KO = 11
VA = KO + 1
NM = R // 4
NC = NM * P
NJ = 64
EPS = 1e-5
SCALE = D ** -0.5

LOG2E = 1.4426950408889634
A10 = 1024.0 * LOG2E
C_TUNED = 61.0

CW = P + P + KO + 32  # ident | wkvB | wq | identB

VSS = 32  # vS slot stride (full transposed kv block)


def _build_nc(c_tuned):
    b10c = 15.0 * 1024.0 - c_tuned
    nc = bacc.Bacc(None, target_bir_lowering=False)

    x_d = nc.dram_tensor("x", [N, D], F32, kind="ExternalInput")
    xq_d = nc.dram_tensor("xq", [NQ, D], F32, kind="ExternalInput")
    cst_d = nc.dram_tensor("consts", [P, CW], F32, kind="ExternalInput")
    y_d = nc.dram_tensor("y", [NQ, KO], F32, kind="ExternalOutput")

    with TileContext(nc) as tc:
        with (
            tc.tile_pool(name="const", bufs=1) as constp,
            tc.tile_pool(name="big", bufs=1) as bigp,
        ):
            eps = constp.tile([P, 1], F32)
            nc.vector.memset(eps, EPS)
            # dummy Sqrt pulls the sqrt table set load to t=0 (under DMAs)
            scr = constp.tile([P, 2], F32)
            nc.scalar.activation(
                out=scr[:, 0:1], in_=eps,
                func=mybir.ActivationFunctionType.Sqrt, bias=0.0, scale=1.0)

            cst = constp.tile([P, CW], F32)
            nc.sync.dma_start(out=cst, in_=cst_d[:])
            ident = cst[:, 0:P]
            identH = constp.tile([P, P], FP16)
            nc.vector.tensor_copy(identH, ident)
            wkvh = constp.tile([P, P], FP16)
            nc.vector.tensor_copy(wkvh, cst[:, P : 2 * P])
            wqh = constp.tile([DA, KO], FP16)
            nc.vector.tensor_copy(wqh, cst[0:DA, 2 * P : 2 * P + KO])

            xnT = bigp.tile([P, NC], FP16)
            kvT = bigp.tile([P, NC], FP16)
            kvT2 = bigp.tile([P, NC], FP16)   # kvT rotated 64 partitions
            qR = bigp.tile([P, NQ], FP16)
            vS = bigp.tile([P, NJ * VSS], FP16)   # transposed kv blocks
            xqT = bigp.tile([DA, NQ], FP16)

            dmaq = [nc.sync, nc.scalar]

            with tc.tile_pool(name="work", bufs=1) as workp:
                pstp_cm = tc.tile_pool(name="pst", bufs=2, space="PSUM")
                pstp = pstp_cm.__enter__()

                def ln_stats(xr, nrows_p, name, sq_on_act=False):
                    """stats chain -> (mu, tenvar=10*var)."""
                    sq = workp.tile([P, nrows_p, D], F32, name=f"sq_{name}")
                    if sq_on_act:
                        nc.scalar.activation(
                            out=sq, in_=xr,
                            func=mybir.ActivationFunctionType.Square,
                            bias=0.0, scale=1.0)
                    else:
                        nc.vector.tensor_mul(sq, xr, xr)
                    s1 = workp.tile([P, nrows_p], F32, name=f"s1_{name}")
                    nc.vector.reduce_sum(out=s1, in_=xr, axis=mybir.AxisListType.X)
                    tv = workp.tile([P, nrows_p], F32, name=f"tv_{name}")
                    nc.vector.reduce_sum(out=tv, in_=sq, axis=mybir.AxisListType.X)
                    mu = workp.tile([P, nrows_p], F32, name=f"mu_{name}")
                    nc.vector.tensor_scalar_mul(mu, s1, 1.0 / D)
                    musq = workp.tile([P, nrows_p], F32, name=f"musq_{name}")
                    nc.vector.tensor_mul(musq, mu, s1)
                    nc.vector.tensor_sub(tv, tv, musq)
                    return mu, tv

                def rsig_of(tv, nrows_p, name):
                    """1/sqrt(tenvar/10 + eps): ACT Sqrt + DVE reciprocal."""
                    sg = workp.tile([P, nrows_p], F32, name=f"sg_{name}")
                    nc.scalar.activation(
                        out=sg, in_=tv,
                        func=mybir.ActivationFunctionType.Sqrt,
                        bias=eps, scale=1.0 / D)
                    rs = workp.tile([P, nrows_p], F32, name=f"rs_{name}")
                    nc.vector.reciprocal(rs, sg)
                    return rs

                def ln_finish(xr, xa_slice, mu, rsig, nrows_p, sub,
                              eng=None):
                    e = eng or nc.vector
                    for h0 in range(0, nrows_p, sub):
                        h1 = min(h0 + sub, nrows_p)
                        nh = h1 - h0
                        e.tensor_sub(
                            xa_slice[:, h0:h1, 0:D], xr[:, h0:h1, :],
                            mu[:, h0:h1].broadcast_to([P, nh, D]),
                        )
                        e.tensor_mul(
                            xa_slice[:, h0:h1, 0:D], xa_slice[:, h0:h1, 0:D],
                            rsig[:, h0:h1].broadcast_to([P, nh, D]),
                        )
                        e.memset(xa_slice[:, h0:h1, D : D + 1], 1.0)

                # ---------- DMAs ----------
                xq_r = workp.tile([P, RQ, D], F32, name="xr_q")
                nc.scalar.dma_start(
                    out=xq_r, in_=xq_d.rearrange("(p r) c -> p r c", p=P)
                )
                x_r = workp.tile([P, R, D], F32, name="xr_x")
                x_src = x_d.rearrange("(p r) c -> p r c", p=P)
                nc.gpsimd.dma_start(out=x_r[:, 0:32, :], in_=x_src[:, 0:32, :])
                nc.sync.dma_start(out=x_r[:, 32:64, :], in_=x_src[:, 32:64, :])

                # ---------- q side ----------
                q_mu, q_tv = ln_stats(xq_r, RQ, "q")
                q_rs = rsig_of(q_tv, RQ, "q")
                xqa = workp.tile([P, RQ, DA], FP16, name="xa_q")
                ln_finish(xq_r, xqa, q_mu, q_rs, RQ, RQ)

                # ---------- x stats (emitted per slice, below) ----------
                xa = workp.tile([P, R, 32], FP16, name="xa_x")
                x_stats = {}

                def x_stats_for(s):
                    xrs = x_r[:, 16 * s : 16 * s + 16, :]
                    mu, tv = ln_stats(xrs, 16, f"x{s}", sq_on_act=True)
                    rs = rsig_of(tv, 16, f"x{s}")
                    x_stats[s] = (mu, rs)

                # ---------- q transposes + projection ----------
                for g in range(RQ // 4):
                    pt = pstp.tile([P, 512], FP16, name="ptq", tag="ps")
                    for k4 in range(4):
                        r = g * 4 + k4
                        nc.tensor.transpose(
                            pt[0:DA, k4 * P : (k4 + 1) * P], xqa[:, r, :],
                            identH,
                        )
                    nc.vector.tensor_copy(
                        xqT[:, g * 512 : (g + 1) * 512], pt[0:DA, :]
                    )
                for t in range(NQ // 512):
                    pq = pstp.tile([P, 512], F32, name="pq", tag="ps")
                    nc.tensor.matmul(
                        pq[0:KO, :], wqh, xqT[:, t * 512 : (t + 1) * 512],
                        start=True, stop=True,
                    )
                    if t % 2 == 0:
                        nc.vector.tensor_copy(
                            qR[0:KO, t * 512 : (t + 1) * 512], pq[0:KO, :])
                    else:
                        nc.scalar.copy(
                            qR[0:KO, t * 512 : (t + 1) * 512], pq[0:KO, :])
                for rp in (32, 64, 96):
                    nc.sync.dma_start(out=qR[rp : rp + KO, :], in_=qR[0:KO, :])

                # explicit sync deps: the DMA-written kvT2/vS consumers race
                # without them (dep tracking misses the strided DMA outputs)
                k2dma = [[None] * 4 for _ in range(4)]
                vdma = [[None] * 4 for _ in range(2)]
                kvcopy = [None] * 4

                # ---------- per-slice normalize/transpose/proj/kvT ------
                def x_slice(s, mk_pt, mk_pk):
                    r0 = 16 * s
                    mu, rs = x_stats[s]
                    xah = xa[:, r0 : r0 + 16, :]
                    # mid-loop slices normalize on the otherwise-idle gpsimd
                    eng = nc.gpsimd if s >= 2 else nc.vector
                    eng.memset(xah[:, :, DA:32], 0.0)
                    ln_finish(x_r[:, r0 : r0 + 16, :], xah, mu, rs, 16, 16,
                              eng=eng)
                    ch = s
                    pt = mk_pt()
                    for mi in range(4):
                        m = ch * 4 + mi
                        nc.tensor.transpose(
                            pt[:, mi * P : (mi + 1) * P],
                            xa[:, m * 4 : m * 4 + 4, :], identH,
                        )
                    dst = xnT[:, ch * 512 : (ch + 1) * 512]
                    if ch % 2 == 0:
                        nc.vector.tensor_copy(dst, pt)
                    else:
                        nc.scalar.copy(dst, pt)
                    pk = mk_pk()
                    nc.tensor.matmul(
                        pk, wkvh, xnT[:, ch * 512 : (ch + 1) * 512],
                        start=True, stop=True,
                    )
                    dstk = kvT[:, ch * 512 : (ch + 1) * 512]
                    if ch % 2 == 0:
                        kvcopy[ch] = nc.scalar.copy(dstk, pk)
                    else:
                        kvcopy[ch] = nc.vector.tensor_copy(dstk, pk)
                    # rotate the k rows by 64 partitions so qk h1 can use
                    # row band (b+2)%4: all 4 qk MMs of a batch run
                    # concurrently. gpsimd DMA queue is idle here.
                    cs = slice(ch * 512, (ch + 1) * 512)
                    for b in range(4):
                        bb = (b + 2) % 4
                        k2dma[ch][bb] = nc.gpsimd.dma_start(
                            out=kvT2[32 * bb : 32 * bb + KO, cs],
                            in_=kvT[32 * b : 32 * b + KO, cs])
                        add_dep_helper(k2dma[ch][bb].ins, kvcopy[ch].ins,
                                       sync=True, reason="kvT2 dma after copy")

                def v_half(h, on_sync):
                    # v row-major: one tiled DMA xbar transpose per base --
                    # in [32, 1024] -> out [128, 8, 32] (8 kv blocks)
                    vS_r4 = vS.rearrange("p (m b c) -> p m b c", b=4, c=VSS)
                    for b in range(4):
                        q = dmaq[0] if on_sync else dmaq[b % 2]
                        vdma[h][b] = q.dma_start_transpose(
                            out=vS_r4[:, h * 8 : (h + 1) * 8, b, :],
                            in_=kvT[32 * b : 32 * b + 32,
                                    h * 1024 : (h + 1) * 1024],
                        )
                        add_dep_helper(vdma[h][b].ins, kvcopy[2 * h].ins,
                                       sync=True, reason="v dma after kv copies")
                        add_dep_helper(vdma[h][b].ins, kvcopy[2 * h + 1].ins,
                                       sync=True, reason="v dma after kv copies")

                mk_pt_p = lambda: pstp.tile([P, 512], FP16, name="ptx", tag="ps")
                mk_pk_p = lambda: pstp.tile([P, 512], F32, name="pk", tag="ps")
                x_stats_for(0)
                x_slice(0, mk_pt_p, mk_pk_p)
                x_stats_for(1)
                x_slice(1, mk_pt_p, mk_pk_p)
                v_half(0, on_sync=False)
                x_stats_for(2)
                x_slice(2, mk_pt_p, mk_pk_p)
                x_stats_for(3)
                x_slice(3, mk_pt_p, mk_pk_p)
                # prefetch the exp table set after the last sqrt
                nc.scalar.activation(
                    out=scr[:, 1:2], in_=x_stats[3][1][:, 0:1],
                    func=mybir.ActivationFunctionType.Exp, bias=0.0, scale=1.0)
                pstp_cm.__exit__(None, None, None)

                # ---------- attention main loop ----------
                bl = [[s, s + 1] for s in range(0, NJ, 2)]
                ACT_EXTRA = set()   # no flips: double-ACT batches stall the recycle
                with tc.tile_pool(name="outp", bufs=1, space="PSUM") as outp:
                    out_big = outp.tile([P, NQ], F32)
                    simp_cm = tc.tile_pool(name="simp", bufs=3, space="PSUM")
                    simp = simp_cm.__enter__()
                    expp_cm = tc.tile_pool(name="expp", bufs=14)
                    expp = expp_cm.__enter__()

                    def emit_av(js, ets, dep):
                        for bi, t in enumerate(js):
                            vj = vS[:, t * VSS + KO : t * VSS + KO + VA]
                            vd = vdma[t // 32][t % 4]
                            for hh in range(NQ // 512):
                                cp = ((t + 2 * hh) % 4) * 32
                                mm = nc.tensor.matmul(
                                    out_big[cp : cp + VA, hh * 512 : (hh + 1) * 512],
                                    vj, ets[bi][:, hh * 512 : (hh + 1) * 512],
                                    start=(t < 2), stop=(t >= NJ - 2),
                                    tile_position=(0, cp),
                                )
                                add_dep_helper(mm.ins, vd.ins, sync=True,
                                               reason="av after v xbar dma")

                    def emit_batch(batch, prev):
                        ets = []
                        last_qk = None
                        for t in batch:
                            m, b = t // 4, t % 4
                            sim = simp.tile([P, NQ], F32, name="sim")
                            for hh in range(NQ // 512):
                                # h1 reads the rotated copy on band (b+2)%4
                                bb = (b + 2 * hh) % 4
                                rp = bb * 32
                                src = kvT if hh == 0 else kvT2
                                last_qk = nc.tensor.matmul(
                                    sim[:, hh * 512 : (hh + 1) * 512],
                                    src[rp : rp + KO, m * P : (m + 1) * P],
                                    qR[rp : rp + KO, hh * 512 : (hh + 1) * 512],
                                    start=True, stop=True,
                                    tile_position=(rp, 0),
                                )
                                if hh == 1:
                                    add_dep_helper(
                                        last_qk.ins, k2dma[t // 16][bb].ins,
                                        sync=True, reason="qk h1 after kvT2 dma")
                            et = expp.tile([P, NQ], FP16, name="et")
                            if t % 2 == 0 or t in ACT_EXTRA:
                                nc.scalar.activation(
                                    out=et, in_=sim,
                                    func=mybir.ActivationFunctionType.Exp,
                                    bias=0.0, scale=1.0,
                                )
                            else:
                                nc.vector.tensor_scalar(
                                    out=et[:].bitcast(I16), in0=sim[:],
                                    scalar1=A10, scalar2=b10c,
                                    op0=mybir.AluOpType.mult,
                                    op1=mybir.AluOpType.add,
                                )
                            ets.append(et)
                        if prev is not None:
                            emit_av(prev[0], prev[1], last_qk)
                        return (batch, ets)

                    mk_pt_l = lambda: simp.tile(
                        [P, NQ], F32, name="sim")[:, 0:256].bitcast(FP16)
                    mk_pk_l = lambda: simp.tile(
                        [P, NQ], F32, name="sim")[:, 0:512]
                    pend = []
                    for bi_, batch in enumerate(bl):
                        if bi_ == 8:
                            v_half(1, on_sync=True)
                        pend.append(emit_batch(batch, None))
                        if len(pend) > 1:
                            b_, e_ = pend.pop(0)
                            emit_av(b_, e_, None)
                    for b_, e_ in pend:
                        emit_av(b_, e_, None)
                    simp_cm.__exit__(None, None, None)
                    expp_cm.__exit__(None, None, None)

                    # ---------- epilogue ----------
                    with tc.tile_pool(name="ep", bufs=1) as epp, \
                         tc.tile_pool(name="epps", bufs=2, space="PSUM") as eppsp:
                        MW = 108
                        oS = epp.tile([P, NQ], F32)
                        nc.vector.tensor_copy(oS[0:MW, 0:512], out_big[0:MW, 0:512])
                        nc.scalar.copy(oS[0:MW, 512:1024], out_big[0:MW, 512:1024])
                        poS = epp.tile([P, RQ * MW], F32)
                        for half in range(2):
                            po = eppsp.tile([P, 4 * MW], F32, name="po")
                            for ti in range(4):
                                t = half * 4 + ti
                                nc.tensor.transpose(
                                    po[:, ti * MW : (ti + 1) * MW],
                                    oS[0:MW, t * P : (t + 1) * P],
                                    ident[0:MW, 0:MW],
                                )
                            dst = poS[:, half * 4 * MW : (half + 1) * 4 * MW]
                            if half == 0:
                                nc.vector.tensor_copy(dst, po)
                            else:
                                nc.scalar.copy(dst, po)
                        poS_r = poS.rearrange("p (t c) -> p t c", c=MW)
                        oM = epp.tile([P, RQ, VA], F32)
                        nc.vector.tensor_add(
                            oM, poS_r[:, :, 0:VA], poS_r[:, :, 32 : 32 + VA])
                        nc.vector.tensor_add(oM, oM, poS_r[:, :, 64 : 64 + VA])
                        nc.vector.tensor_add(oM, oM, poS_r[:, :, 96 : 96 + VA])
                        rec = epp.tile([P, RQ], F32)
                        nc.vector.reciprocal(rec, oM[:, :, KO])
                        oF = epp.tile([P, RQ, KO], F32)
                        nc.vector.tensor_mul(
                            oF, oM[:, :, 0:KO], rec.broadcast_to([P, RQ, KO])
                        )
                        nc.sync.dma_start(
                            out=y_d.rearrange("(p t) c -> p t c", p=P), in_=oF
                        )
    nc.compile()
    return nc


_NC_CACHE = {}


def _get_nc():
    if "nc" not in _NC_CACHE:
        _NC_CACHE["nc"] = _build_nc(C_TUNED)
    return _NC_CACHE["nc"]


def _host_prep(x, gamma, beta, W):
    x = np.asarray(x, np.float32)
    gamma = np.asarray(gamma, np.float32)
    beta = np.asarray(beta, np.float32)
    W = np.asarray(W, np.float32)
    Wg = W * gamma[None, :]
    b0 = W @ beta
    Wq, Wk, Wv = Wg[0:KO], Wg[KO : 2 * KO], Wg[2 * KO : 3 * KO]
    bq, bk, bv = b0[0:KO], b0[KO : 2 * KO], b0[2 * KO : 3 * KO]

    wkvB = np.zeros((P, P), np.float32)
    for b in range(4):
        o = 32 * b
        wkvB[o : o + D, o : o + KO] = Wk.T
        wkvB[o + D, o : o + KO] = bk
        wkvB[o : o + D, o + KO : o + KO + KO] = Wv.T
        wkvB[o + D, o + KO : o + KO + KO] = bv
        wkvB[o + D, o + 2 * KO] = 1.0

    wq_a = np.zeros((DA, KO), np.float32)
    wq_a[0:D, :] = Wq.T * SCALE
    wq_a[D, :] = bq * SCALE

    consts = np.zeros((P, CW), np.float32)
    consts[:, 0:P] = np.eye(P)
    consts[:, P : 2 * P] = wkvB
    consts[0:DA, 2 * P : 2 * P + KO] = wq_a
    o = 2 * P + KO
    for b in range(4):
        consts[32 * b : 32 * b + 32, o : o + 32] = np.eye(32)
    return x, consts


def _run(x, gamma, beta, W, **spmd_kwargs):
    nc = _get_nc()
    x, consts = _host_prep(x, gamma, beta, W)
    in_maps = []
    for c in range(NCORES):
        in_maps.append({
            "x": x,
            "xq": np.ascontiguousarray(x[c * NQ : (c + 1) * NQ]),
            "consts": consts,
        })
    res = run_bass_kernel_spmd(
        nc, in_maps, core_ids=list(range(NCORES)), **spmd_kwargs
    )
    out = np.concatenate([res.results[c]["y"] for c in range(NCORES)], axis=0)
    return out, res


def kernel(x, gamma, beta, W):
    out, _ = _run(x, gamma, beta, W)
    return out

